# revision 10
# baseline (speedup 1.0000x reference)
"""Trainium2 Bass kernel for DeepAngAEVComputer (angular AEV: per-triplet MLP
with weighted per-atom scatter-add).

Contract: kernel(**inputs) takes the FULL unsharded inputs (B=8 molecules) and
returns the FULL [8, 32, 256] output.

Sharding: by ATOM, load-balanced.  Only triplets (i;j,k) with both R_ij and
R_ik inside the 3.5 cutoff contribute (w=0 otherwise); for these inputs that
is ~3.4k of 127k triplets.  The host enumerates surviving triplets per atom,
bin-packs the 256 (molecule, atom) pairs onto 8 cores x 32 output slots
(whole atoms, so the final normalization stays on-device), and pads each
core to T tokens (T=512 default; compile-on-demand ladder up to 16384 for
inputs with more surviving triplets).  The device kernel computes the 9
triplet features, the residual MLP, the cutoff weights and the per-slot
weighted scatter-add + normalization.  Host-side work is only selection /
layout; all reference FLOPs stay on device.

Per-core layout (per chunk of 128*CC tokens; CC=4 for T=512):
  token (a, l): strip a in [0,4), l in [0, 32*CC).  Feature stage holds
  token-major maps FB[32a + l%32, slot, l//32]; the long chem/cosine chain
  runs on the DVE while the independent geo branch + cutoff-min run on the
  GpSimd, and all sqrt/rsqrt are bit-trick + one-Newton-step on the DVE
  (no Sqrt table -> the scalar engine only ever loads sin/tanh tables,
  both off the critical path).  A 32x32 block transpose yields feature-
  major fp16 activations.  The MLP packs two strips per matmul (block-
  stacked / block-diagonal stationary weights) and runs as two half-width
  streams with separate per-half PSUM tiles (avoids false tile-level WARs)
  so each stream's mm->tanh latency hides under the other's engine time;
  residual adds are folded into PSUM accumulation (two matmuls, same
  stationary).  The final 128->256 layer runs token-major: per 128-token
  block, b6 is pre-written into a per-block 2KB PSUM tile by an early
  ones-row K=1 matmul (PE is idle during features), the w6 matmul
  accumulates onto it, tanh reads PSUM directly, and the weighted
  scatter-add is an accumulating [128,32]x[128,256] matmul into a
  persistent [32,256] PSUM tile.  Row normalization: ACT Square+accum for
  the sum of squares, DVE rsqrt, exact (||GA||+eps) reciprocal.

  NB hardware quirks found on the way: tensor_tensor_reduce and APs with
  a count-1 middle dim crash the exec unit (NRT status 101); matmul
  start=True marks its whole 2KB PSUM zero-region pending-zero, so
  start/accumulate pairs must be adjacent in PE order and accumulator
  tiles must own their region.
"""

import os
from contextlib import ExitStack

import ml_dtypes
import numpy as np

import concourse.bass as bass
import concourse.tile as tile
from concourse import bacc
from concourse import mybir
from concourse.bass_utils import run_bass_kernel_spmd

F32 = mybir.dt.float32
F16 = mybir.dt.float16
BF16 = mybir.dt.bfloat16
U32 = mybir.dt.uint32
ALU = mybir.AluOpType
ACTF = mybir.ActivationFunctionType

CUTOFF = 3.5
EPS = 1e-7
CLIP_MIN = 1e-10
PI = float(np.pi)
MAGIC = 0x5F3759DF  # Quake rsqrt seed

NCORE = 8
NA = 32

# ladder of (CC, NCHUNK): T = 128*CC*NCHUNK tokens per core
LADDER = [(4, 1), (8, 1), (16, 1), (16, 2), (16, 4), (16, 8)]

_JI, _KI = np.triu_indices(NA, k=1)

# wpack16 column offsets (fp16 matmul operands + b6 broadcast + ones row)
_W16 = {"w0sA": 0, "w0sB": 128, "w1d": 256, "w2d": 384, "w3d": 512,
        "w4d": 640, "w5p0": 768, "w5p1": 896, "w6": 1024, "b6bc": 1280,
        "ones": 1536}
W16_COLS = 1664
# wf32 columns: per-partition ACT biases
_BIAS = {"b0": 0, "b1": 1, "b2": 2, "b3": 3, "b4": 4, "b5": 5}


# --------------------------------------------------------------------------
# AP helpers
# --------------------------------------------------------------------------

def _ap(base, dims):
    """AP with base's tensor/offset/partition dim and custom free dims."""
    return bass.AP(tensor=base.tensor, offset=base.offset,
                   ap=[list(base.ap[0])] + [list(d) for d in dims])


def slot(fb, i, n=1, step=1, cc=4):
    """[128, n, cc] view of slots i, i+step, ... of FB-like tile [128, S, cc]."""
    base = fb[:, i, :]
    return _ap(base, [[step * cc, n], [1, cc]])


def slot_bc(fb, i, n, cc):
    """slot i broadcast n times along the slot axis."""
    base = fb[:, i, :]
    return _ap(base, [[0, n], [1, cc]])


def slot_T(fb, i, n, cc):
    """[128, cc, n] reordered view (slots innermost) for tensor_reduce."""
    base = fb[:, i, :]
    return _ap(base, [[1, cc], [cc, n]])


# --------------------------------------------------------------------------
# device kernel
# --------------------------------------------------------------------------


def build_kernel(ctx, tc, out_ap, ins, CC, NCHUNK):
    nc = tc.nc
    SL = 32 * CC             # tokens per strip per chunk
    STAGE = int(os.environ.get("AEV_STAGE", "0"))  # debug bisect: 0=full

    def early_exit(src_ap, ncols):
        outs = consts.tile([32, 256], F32, tag="outs", name="outs")
        nc.vector.memset(outs[:], 0.0)
        nc.vector.tensor_copy(outs[:, 0:ncols], src_ap)
        nc.sync.dma_start(out_ap[:], outs[:])

    consts = ctx.enter_context(tc.tile_pool(name="consts", bufs=1))
    fbp = ctx.enter_context(tc.tile_pool(name="fbp", bufs=min(2, NCHUNK)))
    actp = ctx.enter_context(tc.tile_pool(name="actp", bufs=6))
    smal = ctx.enter_context(tc.tile_pool(name="smal", bufs=4))
    psp = ctx.enter_context(
        tc.tile_pool(name="psp", bufs=2, space="PSUM"))
    ps5p = ctx.enter_context(
        tc.tile_pool(name="ps5p", bufs=2, space="PSUM"))
    P6B = 2
    ps6p = ctx.enter_context(
        tc.tile_pool(name="ps6p", bufs=P6B, space="PSUM"))
    gap = ctx.enter_context(tc.tile_pool(name="gap", bufs=1, space="PSUM"))

    # ---- per-chunk workspaces (hoisted so the geom DMA can go first) ----
    FBs, FB2s, fb9s, wts, xb3s, abs_ = [], [], [], [], [], []
    for ch in range(NCHUNK):
        FBs.append(fbp.tile([128, 32, CC], F32, tag="FB", name="FB"))
        FB2s.append(fbp.tile([128, 16, CC], F32, tag="FB2", name="FB2"))
        fb9s.append(fbp.tile([128, CC, 32], F16, tag="fb9", name="fb9"))
        wts.append(fbp.tile([128, 2, CC], F32, tag="wt", name="wt"))
        xb3s.append(fbp.tile([128, 4 * SL], F16, tag="xb3", name="xb3"))
        abs_.append(fbp.tile([128, CC, 32], F16, tag="ab", name="ab"))

    # ---- inputs: geom (features need it) first, then weights / one-hot.
    # Issue the DMAs from FOUR different engine queues so the ~650ns
    # descriptor-write costs overlap instead of serializing on Sync:
    # distances (the first thing the DVE needs) land ~1.3us earlier. ----
    geom_d = ins["geom"][:].rearrange("p (n q c) -> p n q c", n=NCHUNK, q=6)
    nc.sync.dma_start(FBs[0][:, 0:3, :], geom_d[:, 0, 0:3, :])
    nc.gpsimd.dma_start(FBs[0][:, 3:6, :], geom_d[:, 0, 3:6, :])
    wp = consts.tile([128, W16_COLS], F16, tag="wp", name="wp")
    nc.sync.dma_start(wp[:], ins["wpack16"][:])
    oh = consts.tile([128, NCHUNK * CC * 32], BF16, tag="oh", name="oh")
    nc.gpsimd.dma_start(oh[:], ins["oh"][:])
    wb = consts.tile([128, 6], F32, tag="wb", name="wb")
    nc.scalar.dma_start(wb[:], ins["wf32"][:])
    oh_v = oh[:].rearrange("p (n c u) -> p n c u", n=NCHUNK, c=CC)
    for ch in range(1, NCHUNK):
        nc.sync.dma_start(FBs[ch][:, 0:6, :], geom_d[:, ch, :, :])

    magic = consts.tile([128, 1], U32, tag="magic", name="magic")
    nc.vector.memset(magic[:], MAGIC)
    halfpi = consts.tile([128, 1], F32, tag="halfpi", name="halfpi")
    nc.vector.memset(halfpi[:], PI / 2)
    eps32 = consts.tile([32, 1], F32, tag="eps32", name="eps32")
    nc.vector.memset(eps32[:], EPS)
    # Dummy tanh: forces the TANH table-set load to be FIRST in the scalar
    # program (ready ~10us instead of ~12.9us), so the MLP is never
    # table-gated.  The SIN set loads second; its lone use (fc weights)
    # only gates the ga scatter matmuls at ~20us.
    dumt = consts.tile([1, 1], F32, tag="dumt", name="dumt")
    nc.scalar.activation(dumt[:], halfpi[0:1, 0:1], ACTF.Tanh)

    def W(nm, n=128):
        c = _W16[nm]
        return wp[:, c:c + n]

    def B(nm):
        return wb[:, _BIAS[nm]:_BIAS[nm] + 1]

    ga = gap.tile([32, 256], F32, tag="ga", name="ga")

    V = nc.vector

    def mm(ps, w_ap, rhs, start=True, stop=True):
        nc.tensor.matmul(ps, w_ap, rhs, start=start, stop=stop,
                         skip_group_check=True)

    def tanh(dst, src, bias):
        nc.scalar.activation(dst, src, ACTF.Tanh, bias=bias)

    for ch in range(NCHUNK):
        FB, FB2, fb9, wt = FBs[ch], FB2s[ch], fb9s[ch], wts[ch]
        xb3, ab = xb3s[ch], abs_[ch]

        V.memset(fb9[:], 0.0)
        if STAGE == 5:
            return early_exit(FB[0:32, 0:6, :], 6 * CC)

        # final-layer bias: ones-row (K=1) matmuls write b6 into the
        # persistent ps6 tiles up-front -> runs on the PE during the
        # (PE-idle) feature phase, and the L6 tanh reads PSUM directly.
        # Each [128, 512] PAIR tile holds TWO blocks (cols 0:256 / 256:512)
        # so the L6 tanh runs once per pair at [128, 512] width, halving
        # the per-ACTIVATE fixed overhead.  2KB tiles own a full PSUM
        # zero-region (matmul start=True marks the whole region pending-
        # zero), so no false tile-level WAR serializes the final layer.
        def ps6_bias():
            ps6 = ps6p.tile([128, 512], F32, tag="ps6", name="ps6")
            mm(ps6[:, 0:256], wp[0:1, 1536:1664], wp[0:1, 1280:1536],
               start=True, stop=False)
            mm(ps6[:, 256:512], wp[0:1, 1536:1664], wp[0:1, 1280:1536],
               start=False, stop=False)
            return ps6

        NPAIR = CC // 2
        ps6s = [ps6_bias() for _ in range(min(NPAIR, P6B))]

        def S(i, n=1, step=1):
            return slot(FB, i, n, step, CC)

        def S2(i, n=1, step=1):
            return slot(FB2, i, n, step, CC)

        def TT(out, a, b, op):
            V.tensor_tensor(out=out, in0=a, in1=b, op=op)

        # ---- features (slots: 0 rij, 1 rik, 2 rjk, 3 zi, 4 zj, 5 zk).
        # The DVE queue carries only the long chem/cosine dependency chain;
        # the independent geo branch and the cutoff-min run on the otherwise
        # idle GpSimd so they do not stretch the in-order DVE program. ----
        G = nc.gpsimd

        def GT(out, a, b, op):
            G.tensor_tensor(out=out, in0=a, in1=b, op=op)

        TT(S(6, 3), S(0, 3), S(0, 3), ALU.mult)          # sq_ij/ik/jk
        TT(S(9, 2), slot_bc(FB, 0, 2, CC), S(1, 2), ALU.mult)  # p_ijik,p_ijjk
        TT(S(11), S(1), S(2), ALU.mult)                  # p_ikjk
        GT(S(21), S(0), S(1), ALU.add)                   # g0 (gpsimd)
        GT(S(21), S(21), S(2), ALU.add)
        G.tensor_scalar(out=wt[:], in0=S(0, 2), scalar1=CUTOFF,
                        scalar2=None, op0=ALU.min)       # fc args (gpsimd)
        V.tensor_scalar(out=S(12, 3), in0=S(9, 3), scalar1=2.0,
                        scalar2=CLIP_MIN, op0=ALU.mult, op1=ALU.max)
        V.reciprocal(out=S(12, 3), in_=S(12, 3))         # 1/den_i/j/k
        GT(S(22), S(9), S(10), ALU.add)                  # g1 (gpsimd)
        GT(S(22), S(22), S(11), ALU.add)
        GT(S(23), S(9), S(2), ALU.mult)                  # g2 (gpsimd)
        TT(S(15, 2), slot_bc(FB, 6, 2, CC), S(7, 2), ALU.add)
        TT(S(17), S(7), S(8), ALU.add)
        TT(S(15, 3), S(15, 3), S(8, 3, step=-1), ALU.subtract)  # numerators
        TT(S(18, 3), S(15, 3), S(12, 3), ALU.mult)       # c_i, c_j, c_k
        GT(S(12, 3), S(21, 3), S(21, 3), ALU.mult)       # geo sq (gpsimd)
        GT(S(13), S(12), S(13), ALU.add)
        GT(S(30), S(13), S(14), ALU.add)                 # gss (gpsimd)
        TT(S(24, 2), S(4, 2, step=15), S(5, 2, step=15), ALU.add)    # zs, cs
        TT(S(26, 2), S(4, 2, step=15), S(5, 2, step=15), ALU.mult)   # zp, cp
        TT(S(28, 2), S(4, 2, step=15), S(20, 2, step=-15), ALU.mult)  # x-prods
        TT(S(28), S(28), S(29), ALU.add)                 # zc
        TT(S(29), S(26), S(27), ALU.subtract)            # AA
        zic = S(3, 2, step=15)                           # (zi, c_i)
        TT(S2(0, 2), zic, S(24, 2), ALU.add)             # ch0, ch1
        TT(S2(6, 2), zic, S(24, 2), ALU.mult)            # zi*zs, ci*cs
        TT(S2(8, 2), zic, S(25, 2, step=-1), ALU.mult)   # zi*cs, ci*zs
        TT(S2(10, 2), zic, S(29, 2, step=-1), ALU.mult)  # zi*AA, ci*zc
        TT(S2(12, 2), zic, S(28, 2), ALU.mult)           # zi*zc, ci*AA
        TT(S2(2), S2(6), S2(7), ALU.subtract)
        TT(S2(2), S2(2), S(29), ALU.add)                 # ch2
        TT(S2(3), S2(8), S2(9), ALU.add)
        TT(S2(3), S2(3), S(28), ALU.add)                 # ch3
        TT(S2(4), S2(10), S2(11), ALU.subtract)          # ch4
        TT(S2(5), S2(12), S2(13), ALU.add)               # ch5
        # chem sum-of-squares -> slot 31 (geo's slot 30 comes from gpsimd)
        TT(S2(6, 6), S2(0, 6), S2(0, 6), ALU.mult)
        V.tensor_reduce(out=S(31), in_=slot_T(FB2, 6, 6, CC),
                        axis=mybir.AxisListType.X, op=ALU.add)
        nc.scalar.activation(wt[:], wt[:], ACTF.Sin,
                             bias=halfpi[:, 0:1], scale=-PI / CUTOFF)
        # batched rsqrt of (gss, css) -> FB2 slots (14, 15)
        y = S2(14, 2)
        yu = slot(FB2, 14, 2, 1, CC).bitcast(U32)
        su = slot(FB, 30, 2, 1, CC).bitcast(U32)
        V.tensor_scalar(out=yu, in0=su, scalar1=1, scalar2=None,
                        op0=ALU.logical_shift_right)
        TT(yu, _ap(magic[:, 0:1], [[0, 2], [0, CC]]), yu, ALU.subtract)
        t = S(16, 2)
        TT(t, S(30, 2), y, ALU.mult)
        V.tensor_scalar(out=wt[:], in0=wt[:], scalar1=0.5, scalar2=0.5,
                        op0=ALU.mult, op1=ALU.add)       # (fills NR stall)
        TT(t, t, y, ALU.mult)
        TT(wt[:, 0, :], wt[:, 0, :], wt[:, 1, :], ALU.mult)   # w (filler)
        V.tensor_scalar(out=t, in0=t, scalar1=-0.5, scalar2=1.5,
                        op0=ALU.mult, op1=ALU.add)
        TT(y, y, t, ALU.mult)
        # scaled features -> fb9[:, :, 0:9]
        TT(_ap(fb9[:, 0, 0:1], [[1, 3], [32, CC]]), S(21, 3),
           slot_bc(FB2, 14, 3, CC), ALU.mult)
        TT(_ap(fb9[:, 0, 3:4], [[1, 6], [32, CC]]), S2(0, 6),
           slot_bc(FB2, 15, 6, CC), ALU.mult)
        if STAGE == 4:
            return early_exit(fb9[0:32, :, :], 32 * CC)

        # ---- transpose to feature-major (fb9 is already fp16) ----
        V.transpose(out=fb9[:], in_=fb9[:])
        xf = fb9[:].rearrange("p c u -> p (c u)")
        if STAGE == 3:
            return early_exit(fb9[0:32, :, :], 32 * CC)

        # ---- MLP (two strips per matmul via stacked/diag weights).
        # Two half-width streams with SEPARATE per-half PSUM tiles (a
        # shared tile would add a false tile-level WAR: stream B's matmul
        # would wait on stream A's tanh).  Emission interleaves
        # [mmA, tanhA, mmB, tanhB] so A's semaphore is set before mmB
        # occupies the PE queue.  Residual adds are folded into PSUM
        # accumulation: L2 consumes {xres, x1} and L5 consumes {xb1, x4}
        # (xb1 formed off-path on the DVE).
        HA, HB = slice(0, SL), slice(SL, 2 * SL)

        def layer(wname, bias, srcs):
            """One MLP layer: out = tanh(sum_i W^T srcs_i + b), two streams."""
            dst = actp.tile([128, 2 * SL], F16, tag="h", name="h")
            for h in (HA, HB):
                ps = psp.tile([128, SL], F32, tag="ps", name="ps")
                for k, src in enumerate(srcs):
                    mm(ps[:], W(wname), src[:, h], start=(k == 0),
                       stop=(k == len(srcs) - 1))
                tanh(dst[:, h], ps[:], B(bias))
            return dst

        ps0a = psp.tile([128, SL], F32, tag="ps", name="ps")
        mm(ps0a[:], W("w0sA"), xf)
        xres = actp.tile([128, 2 * SL], F16, tag="h", name="h")
        tanh(xres[:, HA], ps0a[:], B("b0"))
        ps0b = psp.tile([128, SL], F32, tag="ps", name="ps")
        mm(ps0b[:], W("w0sB"), xf)
        tanh(xres[:, HB], ps0b[:], B("b0"))
        x1 = layer("w1d", "b1", [xres])
        x2 = layer("w2d", "b2", [xres, x1])
        xb1 = actp.tile([128, 2 * SL], F16, tag="h", name="h")
        TT(xb1[:], x1[:], xres[:], ALU.add)   # off critical path (DVE idle)
        x3 = layer("w3d", "b3", [x2])
        x4 = layer("w4d", "b4", [x3])

        # L5 merged: w5p0 zeroes partitions 64-127, so one 256-col matmul
        # over xb1/x4 yields strips 0 (cols 0:SL) and 1 (cols SL:2SL) at
        # once; w5p1 gives strips 2,3.  One [128, 2*SL] tanh per pair
        # scatters into xb3's block-major column layout.
        def xdst2(a0):
            base = xb3[:, 32 * a0:32 * a0 + 1]
            return _ap(base, [[32, 2], [4 * SL // CC, CC], [1, 32]])

        for w5, a0 in (("w5p0", 0), ("w5p1", 2)):
            ps5 = ps5p.tile([128, 2 * SL], F32, tag="ps5", name="ps5")
            mm(ps5[:], W(w5), xb1[:], start=True, stop=False)
            mm(ps5[:], W(w5), x4[:], start=False, stop=True)
            src = _ap(ps5[:, 0:1], [[SL, 2], [32, CC], [1, 32]])
            tanh(xdst2(a0), src, B("b5"))
        if STAGE == 2:
            return early_exit(xb3[0:32, 0:256], 256)

        # ---- final layer + weighted scatter-add ----
        # All w6 matmuls depend on ALL FOUR L5 tanhs (each xb3 block holds
        # 32 tokens from every strip), so they issue back-to-back once L5
        # finishes; each pair's [128, 512] tanh starts after its two w6
        # matmuls, and the ga scatter matmuls trail the tanhs.
        if STAGE == 7:
            return early_exit(ps6s[0][0:32, 0:256], 256)
        TT(ab[:], oh_v[:, ch, :, :],
           _ap(wt[:, 0, :], [[1, CC], [0, 32]]), ALU.mult)
        otms = []
        for pair in range(NPAIR):
            if pair >= len(ps6s):
                ps6s.append(ps6_bias())
            for sub in range(2):
                bb = 2 * pair + sub
                mm(ps6s[pair][:, 256 * sub:256 * sub + 256],
                   xb3[:, 128 * bb:128 * bb + 128], W("w6", 256),
                   start=False, stop=True)
            if STAGE == 6 and pair == 0:
                return early_exit(ps6s[0][0:32, 0:256], 256)
            otm = smal.tile([128, 512], F16, tag="otm", name="otm")
            nc.scalar.activation(otm[:], ps6s[pair][:], ACTF.Tanh)
            otms.append(otm)

        for pair in range(NPAIR):
            for sub in range(2):
                bb = 2 * pair + sub
                nc.tensor.matmul(
                    ga[:], ab[:, bb, :],
                    otms[pair][:, 256 * sub:256 * sub + 256],
                    start=(ch == 0 and bb == 0),
                    stop=(ch == NCHUNK - 1 and bb == CC - 1),
                    skip_group_check=True)

    if STAGE == 1:
        return early_exit(ga[:], 256)

    # ---- normalize rows of ga, write out (Square is in every ACT table,
    # so the row sum-of-squares costs no table swap and runs on the
    # otherwise-idle scalar engine) ----
    sqj = consts.tile([32, 256], F16, tag="sqj", name="sqj")
    s = consts.tile([32, 4], F32, tag="s", name="s")
    nc.scalar.activation(sqj[:], ga[:], ACTF.Square, accum_out=s[:, 0:1])
    yu = s[:, 1:2].bitcast(U32)
    V.tensor_scalar(out=yu, in0=s[:, 0:1].bitcast(U32), scalar1=1,
                    scalar2=None, op0=ALU.logical_shift_right)
    V.tensor_tensor(out=yu, in0=magic[0:32, 0:1], in1=yu, op=ALU.subtract)
    # one fused Newton step: t = (y*s)*y; y *= 1.5 - 0.5t.  s=0 rows stay
    # exact zeros downstream (y finite, nrm=0, 0 * 1/EPS = 0).
    V.scalar_tensor_tensor(out=s[:, 2:3], in0=s[:, 1:2], scalar=s[:, 0:1],
                           in1=s[:, 1:2], op0=ALU.mult, op1=ALU.mult)
    V.tensor_scalar(out=s[:, 2:3], in0=s[:, 2:3], scalar1=-0.5,
                    scalar2=1.5, op0=ALU.mult, op1=ALU.add)
    # out = (ga * y0) * c  -- the Newton correction y = y0*c is folded into
    # the final scale (c broadcast along the row via a stride-0 AP).
    # y ~ 1/(||ga||+EPS): EPS is negligible against any nonzero ||ga||,
    # and s=0 rows give 0 * finite = 0 exactly.
    # Column halves scale on DVE and ACT in parallel (ACT's free per-
    # partition affine does y0*c in one Copy), each followed by its own
    # half-width DMA issued from that same engine -- two engines write
    # descriptors concurrently and the last output packet lands earlier.
    sc = consts.tile([32, 1], F32, tag="sc", name="sc")
    V.tensor_tensor(out=sc[:], in0=s[:, 1:2], in1=s[:, 2:3], op=ALU.mult)
    outs = consts.tile([32, 256], F32, tag="outs", name="outs")
    nc.scalar.activation(outs[:, 128:256], ga[:, 128:256], ACTF.Copy,
                         bias=0.0, scale=sc[:])
    nc.scalar.dma_start(out_ap[:, 128:256], outs[:, 128:256])
    V.scalar_tensor_tensor(out=outs[:, 0:128], in0=ga[:, 0:128],
                           scalar=s[:, 1:2], in1=_ap(s[:, 2:3], [[0, 128]]),
                           op0=ALU.mult, op1=ALU.mult)
    nc.sync.dma_start(out_ap[:, 0:128], outs[:, 0:128])

# --------------------------------------------------------------------------
# host-side: packing + input prep
# --------------------------------------------------------------------------

def _pack_atoms(cnt, T):
    """LPT bin-pack 256 atoms onto 8 cores (<=32 slots, <=T tokens).

    Returns per-core list of flat atom ids, or None if infeasible."""
    order = np.argsort(-cnt, kind="stable")
    loads = np.zeros(NCORE, np.int64)
    cores = [[] for _ in range(NCORE)]
    for a in order:
        cand = [c for c in range(NCORE) if len(cores[c]) < NA]
        c = min(cand, key=lambda c: loads[c])
        if loads[c] + cnt[a] > T:
            return None
        cores[c].append(int(a))
        loads[c] += cnt[a]
    return cores


def make_inputs(D, S, Ws, bs, CC, NCHUNK, cores, pairs):
    """Build per-core device input dicts."""
    T = 128 * CC * NCHUNK
    SL = 32 * CC

    # shared weight pack (fp16)
    wp = np.zeros((128, W16_COLS), np.float32)
    wp[0:9, 0:64] = Ws[0]
    wp[64:73, 64:128] = Ws[0]
    wp[32:41, 128:192] = Ws[0]
    wp[96:105, 192:256] = Ws[0]
    for i, l in enumerate((1, 2, 3, 4)):
        c = 256 + 128 * i
        wp[0:64, c:c + 64] = Ws[l]
        wp[64:128, c + 64:c + 128] = Ws[l]
    wp[0:64, 768:896] = Ws[5]
    wp[64:128, 896:1024] = Ws[5]
    wp[:, 1024:1280] = Ws[6]
    wp[:, 1280:1536] = np.broadcast_to(bs[6], (128, 256))
    wp[:, 1536:1664] = 1.0
    wp16 = wp.astype(np.float16)

    wf32 = np.zeros((128, 6), np.float32)
    for l in range(5):
        wf32[:, l] = np.concatenate([bs[l], bs[l]])
    wf32[:, 5] = bs[5]

    ins = []
    for c in range(NCORE):
        raw = np.zeros((6, T), np.float32)
        raw[0:2, :] = 5.0
        raw[2:6, :] = 1.0
        slot_of = np.full((T,), -1, np.int64)
        t = 0
        for sidx, a in enumerate(cores[c]):
            b, i = a // NA, a % NA
            for p in pairs[a]:
                j, k = _JI[p], _KI[p]
                raw[0, t] = D[b, i, j]
                raw[1, t] = D[b, i, k]
                raw[2, t] = D[b, j, k]
                raw[3, t] = S[b, i]
                raw[4, t] = S[b, j]
                raw[5, t] = S[b, k]
                slot_of[t] = sidx
                t += 1
        # token t -> (chunk, strip, l): t = TC*ch + SL*a + l;  FB partition
        # row = 32a + l%32, col group = l//32
        tt = np.arange(T)
        ch = tt // (128 * CC)
        a_ = (tt % (128 * CC)) // SL
        l_ = tt % SL
        u = l_ % 32
        cg = l_ // 32
        geom = np.zeros((128, NCHUNK, 6, CC), np.float32)
        geom[32 * a_ + u, ch, :, cg] = raw.T
        oh = np.zeros((128, NCHUNK, CC, 32), np.float32)
        real = slot_of >= 0
        oh[32 * a_[real] + u[real], ch[real], cg[real], slot_of[real]] = 1.0
        ins.append({
            "geom": np.ascontiguousarray(geom.reshape(128, -1)),
            "oh": oh.reshape(128, -1).astype(ml_dtypes.bfloat16),
            "wpack16": wp16,
            "wf32": wf32,
        })
    return ins


# --------------------------------------------------------------------------
# module build + run
# --------------------------------------------------------------------------

_BUILT = {}


def build_bass(CC, NCHUNK):
    key = (CC, NCHUNK)
    if key in _BUILT:
        return _BUILT[key]
    nc = bacc.Bacc(trn_type="TRN2", target_bir_lowering=False, debug=False)
    ins = {
        "geom": nc.dram_tensor("geom", [128, NCHUNK * 6 * CC], F32,
                               kind="ExternalInput").ap(),
        "oh": nc.dram_tensor("oh", [128, NCHUNK * CC * 32], BF16,
                             kind="ExternalInput").ap(),
        "wpack16": nc.dram_tensor("wpack16", [128, W16_COLS], F16,
                                  kind="ExternalInput").ap(),
        "wf32": nc.dram_tensor("wf32", [128, 6], F32,
                               kind="ExternalInput").ap(),
    }
    out = nc.dram_tensor("out", [NA, 256], F32, kind="ExternalOutput").ap()
    with tile.TileContext(nc) as tc:
        with ExitStack() as ctx:
            build_kernel(ctx, tc, out, ins, CC, NCHUNK)
    nc.finalize()
    _BUILT[key] = nc
    return nc


def _run(inputs, **spmd_kwargs):
    D = np.asarray(inputs["distance_matrices"], np.float32)
    S = np.asarray(inputs["num_species_batch"], np.float32)
    Ws = [np.asarray(inputs[f"W{i}"], np.float32) for i in range(7)]
    bs = [np.asarray(inputs[f"b{i}"], np.float32) for i in range(7)]

    # surviving triplets per (molecule, atom)
    cm = (D < CUTOFF) & (D != 0.0)
    m = cm[:, :, _JI] & cm[:, :, _KI]             # [8, 32, 496]
    cnt = m.sum(axis=2).reshape(-1)
    pairs = [np.nonzero(m.reshape(-1, len(_JI))[a])[0] for a in range(8 * NA)]

    for CC, NCHUNK in LADDER:
        T = 128 * CC * NCHUNK
        cores = _pack_atoms(cnt, T)
        if cores is not None:
            break
    else:
        CC, NCHUNK = LADDER[-1]
        cores = [[b * NA + i for i in range(NA)] for b in range(NCORE)]

    nc = build_bass(CC, NCHUNK)
    in_maps = make_inputs(D, S, Ws, bs, CC, NCHUNK, cores, pairs)
    res = run_bass_kernel_spmd(nc, in_maps, core_ids=list(range(NCORE)),
                               **spmd_kwargs)
    out = np.zeros((NCORE, NA, 256), np.float32)
    for c in range(NCORE):
        rc = np.asarray(res.results[c]["out"], np.float32)
        for sidx, a in enumerate(cores[c]):
            out[a // NA, a % NA] = rc[sidx]
    return out, res


def kernel(**inputs):
    out, _ = _run(inputs)
    return out



# revision 11
# speedup vs baseline: 1.2281x; 1.2281x over previous
"""Trainium2 Bass kernel for DeepAngAEVComputer (angular AEV: per-triplet MLP
with weighted per-atom scatter-add).

Contract: kernel(**inputs) takes the FULL unsharded inputs (B=8 molecules) and
returns the FULL [8, 32, 256] output.

Sharding: by ATOM, load-balanced.  Only triplets (i;j,k) with both R_ij and
R_ik inside the 3.5 cutoff contribute (w=0 otherwise); for these inputs that
is ~3.4k of 127k triplets.  The host enumerates surviving triplets per atom,
bin-packs the 256 (molecule, atom) pairs onto 8 cores x 32 output slots
(whole atoms, so the final normalization stays on-device), and pads each
core to T tokens (T=512 default; compile-on-demand ladder up to 16384 for
inputs with more surviving triplets).  The device kernel computes the 9
triplet features, the residual MLP, the cutoff weights and the per-slot
weighted scatter-add + normalization.  Host-side work is only selection /
layout; all reference FLOPs stay on device.

Per-core layout (per chunk of 128*CC tokens; CC=4 for T=512):
  token (a, l): strip a in [0,4), l in [0, 32*CC).  Feature stage holds
  token-major maps FB[32a + l%32, slot, l//32]; the long chem/cosine chain
  runs on the DVE while the independent geo branch + cutoff-min run on the
  GpSimd, and all sqrt/rsqrt are bit-trick + one-Newton-step on the DVE
  (no Sqrt table -> the scalar engine only ever loads sin/tanh tables,
  both off the critical path).  A 32x32 block transpose yields feature-
  major fp16 activations.  The MLP packs two strips per matmul (block-
  stacked / block-diagonal stationary weights) and runs as two half-width
  streams with separate per-half PSUM tiles (avoids false tile-level WARs)
  so each stream's mm->tanh latency hides under the other's engine time;
  residual adds are folded into PSUM accumulation (two matmuls, same
  stationary).  The final 128->256 layer runs token-major: per 128-token
  block, b6 is pre-written into a per-block 2KB PSUM tile by an early
  ones-row K=1 matmul (PE is idle during features), the w6 matmul
  accumulates onto it, tanh reads PSUM directly, and the weighted
  scatter-add is an accumulating [128,32]x[128,256] matmul into a
  persistent [32,256] PSUM tile.  Row normalization: ACT Square+accum for
  the sum of squares, DVE rsqrt, exact (||GA||+eps) reciprocal.

  NB hardware quirks found on the way: tensor_tensor_reduce and APs with
  a count-1 middle dim crash the exec unit (NRT status 101); matmul
  start=True marks its whole 2KB PSUM zero-region pending-zero, so
  start/accumulate pairs must be adjacent in PE order and accumulator
  tiles must own their region.
"""

import os
from contextlib import ExitStack

import ml_dtypes
import numpy as np

import concourse.bass as bass
import concourse.tile as tile
from concourse import bacc
from concourse import mybir
from concourse.bass_utils import run_bass_kernel_spmd

F32 = mybir.dt.float32
F16 = mybir.dt.float16
BF16 = mybir.dt.bfloat16
U32 = mybir.dt.uint32
ALU = mybir.AluOpType
ACTF = mybir.ActivationFunctionType

CUTOFF = 3.5
EPS = 1e-7
CLIP_MIN = 1e-10
PI = float(np.pi)
MAGIC = 0x5F3759DF  # Quake rsqrt seed

NCORE = 8
NA = 32

# ladder of (CC, NCHUNK): T = 128*CC*NCHUNK tokens per core
LADDER = [(4, 1), (8, 1), (16, 1), (16, 2), (16, 4), (16, 8)]

_JI, _KI = np.triu_indices(NA, k=1)

# wpack16 column offsets (fp16 matmul operands + b6 broadcast + ones row)
_W16 = {"w0sA": 0, "w0sB": 128, "w1d": 256, "w2d": 384, "w3d": 512,
        "w4d": 640, "w5p0": 768, "w5p1": 896, "w6": 1024, "b6bc": 1280,
        "ones": 1536}
W16_COLS = 1664
# wf32 columns: per-partition ACT biases
_BIAS = {"b0": 0, "b1": 1, "b2": 2, "b3": 3, "b4": 4, "b5": 5}


# --------------------------------------------------------------------------
# AP helpers
# --------------------------------------------------------------------------

def _ap(base, dims):
    """AP with base's tensor/offset/partition dim and custom free dims."""
    return bass.AP(tensor=base.tensor, offset=base.offset,
                   ap=[list(base.ap[0])] + [list(d) for d in dims])


def slot(fb, i, n=1, step=1, cc=4):
    """[128, n, cc] view of slots i, i+step, ... of FB-like tile [128, S, cc]."""
    base = fb[:, i, :]
    return _ap(base, [[step * cc, n], [1, cc]])


def slot_bc(fb, i, n, cc):
    """slot i broadcast n times along the slot axis."""
    base = fb[:, i, :]
    return _ap(base, [[0, n], [1, cc]])


def slot_T(fb, i, n, cc):
    """[128, cc, n] reordered view (slots innermost) for tensor_reduce."""
    base = fb[:, i, :]
    return _ap(base, [[1, cc], [cc, n]])


# --------------------------------------------------------------------------
# device kernel
# --------------------------------------------------------------------------


def build_kernel(ctx, tc, out_ap, ins, CC, NCHUNK):
    nc = tc.nc
    SL = 32 * CC             # tokens per strip per chunk
    STAGE = int(os.environ.get("AEV_STAGE", "0"))  # debug bisect: 0=full

    def early_exit(src_ap, ncols):
        outs = consts.tile([32, 256], F32, tag="outs", name="outs")
        nc.vector.memset(outs[:], 0.0)
        nc.vector.tensor_copy(outs[:, 0:ncols], src_ap)
        nc.sync.dma_start(out_ap[:], outs[:])

    consts = ctx.enter_context(tc.tile_pool(name="consts", bufs=1))
    fbp = ctx.enter_context(tc.tile_pool(name="fbp", bufs=min(2, NCHUNK)))
    actp = ctx.enter_context(tc.tile_pool(name="actp", bufs=6))
    smal = ctx.enter_context(tc.tile_pool(name="smal", bufs=4))
    psp = ctx.enter_context(
        tc.tile_pool(name="psp", bufs=2, space="PSUM"))
    ps5p = ctx.enter_context(
        tc.tile_pool(name="ps5p", bufs=2, space="PSUM"))
    P6B = 2
    ps6p = ctx.enter_context(
        tc.tile_pool(name="ps6p", bufs=P6B, space="PSUM"))
    gap = ctx.enter_context(tc.tile_pool(name="gap", bufs=1, space="PSUM"))

    # ---- per-chunk workspaces (hoisted so the geom DMA can go first) ----
    FBs, FB2s, fb9s, wts, xb3s, abs_ = [], [], [], [], [], []
    for ch in range(NCHUNK):
        FBs.append(fbp.tile([128, 32, CC], F32, tag="FB", name="FB"))
        FB2s.append(fbp.tile([128, 16, CC], F32, tag="FB2", name="FB2"))
        fb9s.append(fbp.tile([128, CC, 32], F16, tag="fb9", name="fb9"))
        wts.append(fbp.tile([128, 2, CC], F32, tag="wt", name="wt"))
        xb3s.append(fbp.tile([128, 4 * SL], F16, tag="xb3", name="xb3"))
        abs_.append(fbp.tile([128, CC, 32], F16, tag="ab", name="ab"))

    # ---- inputs: geom (features need it) first, then weights / one-hot.
    # wpack16 is second on the sync queue (it gates the up-front bias
    # matmuls and the L0 weights); oh issues from the gpsimd queue in
    # parallel.  NB: the scalar-engine table-set allocator reloads on
    # every set SWITCH in program order, so the scalar program must stay
    # [sin..., tanh...] -- exactly two loads. ----
    geom_d = ins["geom"][:].rearrange("p (n q c) -> p n q c", n=NCHUNK, q=6)
    nc.sync.dma_start(FBs[0][:, 0:6, :], geom_d[:, 0, :, :])
    wp = consts.tile([128, W16_COLS], F16, tag="wp", name="wp")
    nc.sync.dma_start(wp[:], ins["wpack16"][:])
    wb = consts.tile([128, 6], F32, tag="wb", name="wb")
    nc.sync.dma_start(wb[:], ins["wf32"][:])
    oh = consts.tile([128, NCHUNK * CC * 32], BF16, tag="oh", name="oh")
    nc.gpsimd.dma_start(oh[:], ins["oh"][:])
    oh_v = oh[:].rearrange("p (n c u) -> p n c u", n=NCHUNK, c=CC)
    for ch in range(1, NCHUNK):
        nc.sync.dma_start(FBs[ch][:, 0:6, :], geom_d[:, ch, :, :])

    magic = consts.tile([128, 1], U32, tag="magic", name="magic")
    nc.vector.memset(magic[:], MAGIC)
    halfpi = consts.tile([128, 1], F32, tag="halfpi", name="halfpi")
    nc.vector.memset(halfpi[:], PI / 2)
    eps32 = consts.tile([32, 1], F32, tag="eps32", name="eps32")
    nc.vector.memset(eps32[:], EPS)

    def W(nm, n=128):
        c = _W16[nm]
        return wp[:, c:c + n]

    def B(nm):
        return wb[:, _BIAS[nm]:_BIAS[nm] + 1]

    ga = gap.tile([32, 256], F32, tag="ga", name="ga")

    V = nc.vector

    def mm(ps, w_ap, rhs, start=True, stop=True):
        nc.tensor.matmul(ps, w_ap, rhs, start=start, stop=stop,
                         skip_group_check=True)

    def tanh(dst, src, bias):
        nc.scalar.activation(dst, src, ACTF.Tanh, bias=bias)

    for ch in range(NCHUNK):
        FB, FB2, fb9, wt = FBs[ch], FB2s[ch], fb9s[ch], wts[ch]
        xb3, ab = xb3s[ch], abs_[ch]

        V.memset(fb9[:], 0.0)
        if STAGE == 5:
            return early_exit(FB[0:32, 0:6, :], 6 * CC)

        # final-layer bias: ones-row (K=1) matmuls write b6 into the
        # persistent ps6 tiles up-front -> runs on the PE during the
        # (PE-idle) feature phase, and the L6 tanh reads PSUM directly.
        # Each [128, 512] PAIR tile holds TWO blocks (cols 0:256 / 256:512)
        # so the L6 tanh runs once per pair at [128, 512] width, halving
        # the per-ACTIVATE fixed overhead.  2KB tiles own a full PSUM
        # zero-region (matmul start=True marks the whole region pending-
        # zero), so no false tile-level WAR serializes the final layer.
        def ps6_bias():
            ps6 = ps6p.tile([128, 512], F32, tag="ps6", name="ps6")
            mm(ps6[:, 0:256], wp[0:1, 1536:1664], wp[0:1, 1280:1536],
               start=True, stop=False)
            mm(ps6[:, 256:512], wp[0:1, 1536:1664], wp[0:1, 1280:1536],
               start=False, stop=False)
            return ps6

        NPAIR = CC // 2
        ps6s = [ps6_bias() for _ in range(min(NPAIR, P6B))]

        def S(i, n=1, step=1):
            return slot(FB, i, n, step, CC)

        def S2(i, n=1, step=1):
            return slot(FB2, i, n, step, CC)

        def TT(out, a, b, op):
            V.tensor_tensor(out=out, in0=a, in1=b, op=op)

        # ---- features (slots: 0 rij, 1 rik, 2 rjk, 3 zi, 4 zj, 5 zk).
        # The DVE queue carries only the long chem/cosine dependency chain;
        # the independent geo branch and the cutoff-min run on the otherwise
        # idle GpSimd so they do not stretch the in-order DVE program. ----
        G = nc.gpsimd

        def GT(out, a, b, op):
            G.tensor_tensor(out=out, in0=a, in1=b, op=op)

        TT(S(6, 3), S(0, 3), S(0, 3), ALU.mult)          # sq_ij/ik/jk
        TT(S(9, 2), slot_bc(FB, 0, 2, CC), S(1, 2), ALU.mult)  # p_ijik,p_ijjk
        TT(S(11), S(1), S(2), ALU.mult)                  # p_ikjk
        GT(S(21), S(0), S(1), ALU.add)                   # g0 (gpsimd)
        GT(S(21), S(21), S(2), ALU.add)
        G.tensor_scalar(out=wt[:], in0=S(0, 2), scalar1=CUTOFF,
                        scalar2=None, op0=ALU.min)       # fc args (gpsimd)
        V.tensor_scalar(out=S(12, 3), in0=S(9, 3), scalar1=2.0,
                        scalar2=CLIP_MIN, op0=ALU.mult, op1=ALU.max)
        V.reciprocal(out=S(12, 3), in_=S(12, 3))         # 1/den_i/j/k
        GT(S(22), S(9), S(10), ALU.add)                  # g1 (gpsimd)
        GT(S(22), S(22), S(11), ALU.add)
        GT(S(23), S(9), S(2), ALU.mult)                  # g2 (gpsimd)
        TT(S(15, 2), slot_bc(FB, 6, 2, CC), S(7, 2), ALU.add)
        TT(S(17), S(7), S(8), ALU.add)
        TT(S(15, 3), S(15, 3), S(8, 3, step=-1), ALU.subtract)  # numerators
        TT(S(18, 3), S(15, 3), S(12, 3), ALU.mult)       # c_i, c_j, c_k
        GT(S(12, 3), S(21, 3), S(21, 3), ALU.mult)       # geo sq (gpsimd)
        GT(S(13), S(12), S(13), ALU.add)
        GT(S(30), S(13), S(14), ALU.add)                 # gss (gpsimd)
        TT(S(24, 2), S(4, 2, step=15), S(5, 2, step=15), ALU.add)    # zs, cs
        TT(S(26, 2), S(4, 2, step=15), S(5, 2, step=15), ALU.mult)   # zp, cp
        TT(S(28, 2), S(4, 2, step=15), S(20, 2, step=-15), ALU.mult)  # x-prods
        TT(S(28), S(28), S(29), ALU.add)                 # zc
        TT(S(29), S(26), S(27), ALU.subtract)            # AA
        zic = S(3, 2, step=15)                           # (zi, c_i)
        TT(S2(0, 2), zic, S(24, 2), ALU.add)             # ch0, ch1
        TT(S2(6, 2), zic, S(24, 2), ALU.mult)            # zi*zs, ci*cs
        TT(S2(8, 2), zic, S(25, 2, step=-1), ALU.mult)   # zi*cs, ci*zs
        TT(S2(10, 2), zic, S(29, 2, step=-1), ALU.mult)  # zi*AA, ci*zc
        TT(S2(12, 2), zic, S(28, 2), ALU.mult)           # zi*zc, ci*AA
        TT(S2(2), S2(6), S2(7), ALU.subtract)
        TT(S2(2), S2(2), S(29), ALU.add)                 # ch2
        TT(S2(3), S2(8), S2(9), ALU.add)
        TT(S2(3), S2(3), S(28), ALU.add)                 # ch3
        TT(S2(4), S2(10), S2(11), ALU.subtract)          # ch4
        TT(S2(5), S2(12), S2(13), ALU.add)               # ch5
        # chem sum-of-squares -> slot 31 (geo's slot 30 comes from gpsimd)
        TT(S2(6, 6), S2(0, 6), S2(0, 6), ALU.mult)
        V.tensor_reduce(out=S(31), in_=slot_T(FB2, 6, 6, CC),
                        axis=mybir.AxisListType.X, op=ALU.add)
        nc.scalar.activation(wt[:], wt[:], ACTF.Sin,
                             bias=halfpi[:, 0:1], scale=-PI / CUTOFF)
        # batched rsqrt of (gss, css) -> FB2 slots (14, 15)
        y = S2(14, 2)
        yu = slot(FB2, 14, 2, 1, CC).bitcast(U32)
        su = slot(FB, 30, 2, 1, CC).bitcast(U32)
        V.tensor_scalar(out=yu, in0=su, scalar1=1, scalar2=None,
                        op0=ALU.logical_shift_right)
        TT(yu, _ap(magic[:, 0:1], [[0, 2], [0, CC]]), yu, ALU.subtract)
        t = S(16, 2)
        TT(t, S(30, 2), y, ALU.mult)
        V.tensor_scalar(out=wt[:], in0=wt[:], scalar1=0.5, scalar2=0.5,
                        op0=ALU.mult, op1=ALU.add)       # (fills NR stall)
        TT(t, t, y, ALU.mult)
        TT(wt[:, 0, :], wt[:, 0, :], wt[:, 1, :], ALU.mult)   # w (filler)
        V.tensor_scalar(out=t, in0=t, scalar1=-0.5, scalar2=1.5,
                        op0=ALU.mult, op1=ALU.add)
        TT(y, y, t, ALU.mult)
        # scaled features -> fb9[:, :, 0:9]
        TT(_ap(fb9[:, 0, 0:1], [[1, 3], [32, CC]]), S(21, 3),
           slot_bc(FB2, 14, 3, CC), ALU.mult)
        TT(_ap(fb9[:, 0, 3:4], [[1, 6], [32, CC]]), S2(0, 6),
           slot_bc(FB2, 15, 6, CC), ALU.mult)
        if STAGE == 4:
            return early_exit(fb9[0:32, :, :], 32 * CC)

        # ---- transpose to feature-major (fb9 is already fp16) ----
        V.transpose(out=fb9[:], in_=fb9[:])
        xf = fb9[:].rearrange("p c u -> p (c u)")
        if STAGE == 3:
            return early_exit(fb9[0:32, :, :], 32 * CC)

        # ---- MLP (two strips per matmul via stacked/diag weights).
        # Two half-width streams with SEPARATE per-half PSUM tiles (a
        # shared tile would add a false tile-level WAR: stream B's matmul
        # would wait on stream A's tanh).  Emission interleaves
        # [mmA, tanhA, mmB, tanhB] so A's semaphore is set before mmB
        # occupies the PE queue.  Residual adds are folded into PSUM
        # accumulation: L2 consumes {xres, x1} and L5 consumes {xb1, x4}
        # (xb1 formed off-path on the DVE).
        HA, HB = slice(0, SL), slice(SL, 2 * SL)

        def layer(wname, bias, srcs):
            """One MLP layer: out = tanh(sum_i W^T srcs_i + b), two streams."""
            dst = actp.tile([128, 2 * SL], F16, tag="h", name="h")
            for h in (HA, HB):
                ps = psp.tile([128, SL], F32, tag="ps", name="ps")
                for k, src in enumerate(srcs):
                    mm(ps[:], W(wname), src[:, h], start=(k == 0),
                       stop=(k == len(srcs) - 1))
                tanh(dst[:, h], ps[:], B(bias))
            return dst

        ps0a = psp.tile([128, SL], F32, tag="ps", name="ps")
        mm(ps0a[:], W("w0sA"), xf)
        xres = actp.tile([128, 2 * SL], F16, tag="h", name="h")
        tanh(xres[:, HA], ps0a[:], B("b0"))
        ps0b = psp.tile([128, SL], F32, tag="ps", name="ps")
        mm(ps0b[:], W("w0sB"), xf)
        tanh(xres[:, HB], ps0b[:], B("b0"))
        x1 = layer("w1d", "b1", [xres])
        x2 = layer("w2d", "b2", [xres, x1])
        xb1 = actp.tile([128, 2 * SL], F16, tag="h", name="h")
        TT(xb1[:], x1[:], xres[:], ALU.add)   # off critical path (DVE idle)
        x3 = layer("w3d", "b3", [x2])
        x4 = layer("w4d", "b4", [x3])

        # L5 merged: w5p0 zeroes partitions 64-127, so one 256-col matmul
        # over xb1/x4 yields strips 0 (cols 0:SL) and 1 (cols SL:2SL) at
        # once; w5p1 gives strips 2,3.  One [128, 2*SL] tanh per pair
        # scatters into xb3's block-major column layout.
        def xdst2(a0):
            base = xb3[:, 32 * a0:32 * a0 + 1]
            return _ap(base, [[32, 2], [4 * SL // CC, CC], [1, 32]])

        for w5, a0 in (("w5p0", 0), ("w5p1", 2)):
            ps5 = ps5p.tile([128, 2 * SL], F32, tag="ps5", name="ps5")
            mm(ps5[:], W(w5), xb1[:], start=True, stop=False)
            mm(ps5[:], W(w5), x4[:], start=False, stop=True)
            src = _ap(ps5[:, 0:1], [[SL, 2], [32, CC], [1, 32]])
            tanh(xdst2(a0), src, B("b5"))
        if STAGE == 2:
            return early_exit(xb3[0:32, 0:256], 256)

        # ---- final layer + weighted scatter-add ----
        # All w6 matmuls depend on ALL FOUR L5 tanhs (each xb3 block holds
        # 32 tokens from every strip), so they issue back-to-back once L5
        # finishes; each pair's [128, 512] tanh starts after its two w6
        # matmuls, and the ga scatter matmuls trail the tanhs.
        if STAGE == 7:
            return early_exit(ps6s[0][0:32, 0:256], 256)
        TT(ab[:], oh_v[:, ch, :, :],
           _ap(wt[:, 0, :], [[1, CC], [0, 32]]), ALU.mult)
        otms = []
        for pair in range(NPAIR):
            if pair >= len(ps6s):
                ps6s.append(ps6_bias())
            for sub in range(2):
                bb = 2 * pair + sub
                mm(ps6s[pair][:, 256 * sub:256 * sub + 256],
                   xb3[:, 128 * bb:128 * bb + 128], W("w6", 256),
                   start=False, stop=True)
            if STAGE == 6 and pair == 0:
                return early_exit(ps6s[0][0:32, 0:256], 256)
            otm = smal.tile([128, 512], F16, tag="otm", name="otm")
            nc.scalar.activation(otm[:], ps6s[pair][:], ACTF.Tanh)
            otms.append(otm)

        for pair in range(NPAIR):
            for sub in range(2):
                bb = 2 * pair + sub
                nc.tensor.matmul(
                    ga[:], ab[:, bb, :],
                    otms[pair][:, 256 * sub:256 * sub + 256],
                    start=(ch == 0 and bb == 0),
                    stop=(ch == NCHUNK - 1 and bb == CC - 1),
                    skip_group_check=True)

    if STAGE == 1:
        return early_exit(ga[:], 256)

    # ---- normalize rows of ga, write out (Square is in every ACT table,
    # so the row sum-of-squares costs no table swap and runs on the
    # otherwise-idle scalar engine) ----
    sqj = consts.tile([32, 256], F16, tag="sqj", name="sqj")
    s = consts.tile([32, 4], F32, tag="s", name="s")
    nc.scalar.activation(sqj[:], ga[:], ACTF.Square, accum_out=s[:, 0:1])
    yu = s[:, 1:2].bitcast(U32)
    V.tensor_scalar(out=yu, in0=s[:, 0:1].bitcast(U32), scalar1=1,
                    scalar2=None, op0=ALU.logical_shift_right)
    V.tensor_tensor(out=yu, in0=magic[0:32, 0:1], in1=yu, op=ALU.subtract)
    # one fused Newton step: t = (y*s)*y; y *= 1.5 - 0.5t.  s=0 rows stay
    # exact zeros downstream (y finite, nrm=0, 0 * 1/EPS = 0).
    V.scalar_tensor_tensor(out=s[:, 2:3], in0=s[:, 1:2], scalar=s[:, 0:1],
                           in1=s[:, 1:2], op0=ALU.mult, op1=ALU.mult)
    V.tensor_scalar(out=s[:, 2:3], in0=s[:, 2:3], scalar1=-0.5,
                    scalar2=1.5, op0=ALU.mult, op1=ALU.add)
    # out = (ga * y0) * c  -- the Newton correction y = y0*c is folded into
    # the final scale (c broadcast along the row via a stride-0 AP).
    # y ~ 1/(||ga||+EPS): EPS is negligible against any nonzero ||ga||,
    # and s=0 rows give 0 * finite = 0 exactly.
    # Column halves scale on DVE and ACT in parallel (ACT's free per-
    # partition affine does y0*c in one Copy), each followed by its own
    # half-width DMA issued from that same engine -- two engines write
    # descriptors concurrently and the last output packet lands earlier.
    sc = consts.tile([32, 1], F32, tag="sc", name="sc")
    V.tensor_tensor(out=sc[:], in0=s[:, 1:2], in1=s[:, 2:3], op=ALU.mult)
    outs = consts.tile([32, 256], F32, tag="outs", name="outs")
    nc.scalar.activation(outs[:, 128:256], ga[:, 128:256], ACTF.Copy,
                         bias=0.0, scale=sc[:])
    nc.scalar.dma_start(out_ap[:, 128:256], outs[:, 128:256])
    V.scalar_tensor_tensor(out=outs[:, 0:128], in0=ga[:, 0:128],
                           scalar=s[:, 1:2], in1=_ap(s[:, 2:3], [[0, 128]]),
                           op0=ALU.mult, op1=ALU.mult)
    nc.sync.dma_start(out_ap[:, 0:128], outs[:, 0:128])

# --------------------------------------------------------------------------
# host-side: packing + input prep
# --------------------------------------------------------------------------

def _pack_atoms(cnt, T):
    """LPT bin-pack 256 atoms onto 8 cores (<=32 slots, <=T tokens).

    Returns per-core list of flat atom ids, or None if infeasible."""
    order = np.argsort(-cnt, kind="stable")
    loads = np.zeros(NCORE, np.int64)
    cores = [[] for _ in range(NCORE)]
    for a in order:
        cand = [c for c in range(NCORE) if len(cores[c]) < NA]
        c = min(cand, key=lambda c: loads[c])
        if loads[c] + cnt[a] > T:
            return None
        cores[c].append(int(a))
        loads[c] += cnt[a]
    return cores


def make_inputs(D, S, Ws, bs, CC, NCHUNK, cores, pairs):
    """Build per-core device input dicts."""
    T = 128 * CC * NCHUNK
    SL = 32 * CC

    # shared weight pack (fp16)
    wp = np.zeros((128, W16_COLS), np.float32)
    wp[0:9, 0:64] = Ws[0]
    wp[64:73, 64:128] = Ws[0]
    wp[32:41, 128:192] = Ws[0]
    wp[96:105, 192:256] = Ws[0]
    for i, l in enumerate((1, 2, 3, 4)):
        c = 256 + 128 * i
        wp[0:64, c:c + 64] = Ws[l]
        wp[64:128, c + 64:c + 128] = Ws[l]
    wp[0:64, 768:896] = Ws[5]
    wp[64:128, 896:1024] = Ws[5]
    wp[:, 1024:1280] = Ws[6]
    wp[:, 1280:1536] = np.broadcast_to(bs[6], (128, 256))
    wp[:, 1536:1664] = 1.0
    wp16 = wp.astype(np.float16)

    wf32 = np.zeros((128, 6), np.float32)
    for l in range(5):
        wf32[:, l] = np.concatenate([bs[l], bs[l]])
    wf32[:, 5] = bs[5]

    ins = []
    for c in range(NCORE):
        raw = np.zeros((6, T), np.float32)
        raw[0:2, :] = 5.0
        raw[2:6, :] = 1.0
        slot_of = np.full((T,), -1, np.int64)
        t = 0
        for sidx, a in enumerate(cores[c]):
            b, i = a // NA, a % NA
            for p in pairs[a]:
                j, k = _JI[p], _KI[p]
                raw[0, t] = D[b, i, j]
                raw[1, t] = D[b, i, k]
                raw[2, t] = D[b, j, k]
                raw[3, t] = S[b, i]
                raw[4, t] = S[b, j]
                raw[5, t] = S[b, k]
                slot_of[t] = sidx
                t += 1
        # token t -> (chunk, strip, l): t = TC*ch + SL*a + l;  FB partition
        # row = 32a + l%32, col group = l//32
        tt = np.arange(T)
        ch = tt // (128 * CC)
        a_ = (tt % (128 * CC)) // SL
        l_ = tt % SL
        u = l_ % 32
        cg = l_ // 32
        geom = np.zeros((128, NCHUNK, 6, CC), np.float32)
        geom[32 * a_ + u, ch, :, cg] = raw.T
        oh = np.zeros((128, NCHUNK, CC, 32), np.float32)
        real = slot_of >= 0
        oh[32 * a_[real] + u[real], ch[real], cg[real], slot_of[real]] = 1.0
        ins.append({
            "geom": np.ascontiguousarray(geom.reshape(128, -1)),
            "oh": oh.reshape(128, -1).astype(ml_dtypes.bfloat16),
            "wpack16": wp16,
            "wf32": wf32,
        })
    return ins


# --------------------------------------------------------------------------
# module build + run
# --------------------------------------------------------------------------

_BUILT = {}


def build_bass(CC, NCHUNK):
    key = (CC, NCHUNK)
    if key in _BUILT:
        return _BUILT[key]
    nc = bacc.Bacc(trn_type="TRN2", target_bir_lowering=False, debug=False)
    ins = {
        "geom": nc.dram_tensor("geom", [128, NCHUNK * 6 * CC], F32,
                               kind="ExternalInput").ap(),
        "oh": nc.dram_tensor("oh", [128, NCHUNK * CC * 32], BF16,
                             kind="ExternalInput").ap(),
        "wpack16": nc.dram_tensor("wpack16", [128, W16_COLS], F16,
                                  kind="ExternalInput").ap(),
        "wf32": nc.dram_tensor("wf32", [128, 6], F32,
                               kind="ExternalInput").ap(),
    }
    out = nc.dram_tensor("out", [NA, 256], F32, kind="ExternalOutput").ap()
    with tile.TileContext(nc) as tc:
        with ExitStack() as ctx:
            build_kernel(ctx, tc, out, ins, CC, NCHUNK)
    nc.finalize()
    _BUILT[key] = nc
    return nc


def _run(inputs, **spmd_kwargs):
    D = np.asarray(inputs["distance_matrices"], np.float32)
    S = np.asarray(inputs["num_species_batch"], np.float32)
    Ws = [np.asarray(inputs[f"W{i}"], np.float32) for i in range(7)]
    bs = [np.asarray(inputs[f"b{i}"], np.float32) for i in range(7)]

    # surviving triplets per (molecule, atom)
    cm = (D < CUTOFF) & (D != 0.0)
    m = cm[:, :, _JI] & cm[:, :, _KI]             # [8, 32, 496]
    cnt = m.sum(axis=2).reshape(-1)
    pairs = [np.nonzero(m.reshape(-1, len(_JI))[a])[0] for a in range(8 * NA)]

    for CC, NCHUNK in LADDER:
        T = 128 * CC * NCHUNK
        cores = _pack_atoms(cnt, T)
        if cores is not None:
            break
    else:
        CC, NCHUNK = LADDER[-1]
        cores = [[b * NA + i for i in range(NA)] for b in range(NCORE)]

    nc = build_bass(CC, NCHUNK)
    in_maps = make_inputs(D, S, Ws, bs, CC, NCHUNK, cores, pairs)
    res = run_bass_kernel_spmd(nc, in_maps, core_ids=list(range(NCORE)),
                               **spmd_kwargs)
    out = np.zeros((NCORE, NA, 256), np.float32)
    for c in range(NCORE):
        rc = np.asarray(res.results[c]["out"], np.float32)
        for sidx, a in enumerate(cores[c]):
            out[a // NA, a % NA] = rc[sidx]
    return out, res


def kernel(**inputs):
    out, _ = _run(inputs)
    return out

